# revision 2
# baseline (speedup 1.0000x reference)
"""Block-sparse matmul (sparse_attention) Trainium2 kernel.

y[:, j*32:(j+1)*32] += x[:, i*32:(i+1)*32] @ w[b]   for each sparse block b=(i,j)

Sharding: data-parallel over M rows (8 cores x 512 rows), w + indices
replicated, no collectives.  Per core we compute y^T:
    yT[j-block, :] += w[b].T @ xT[i-block, :]
using the PE 32x32 tiling mode: block (i,j) runs on PE tile
(row-group i%4, col-group j%4); rhs = xT tile t=i//4 rows 32*(i%4).. ;
output accumulates in a PSUM bank dedicated to row-lane r=i%4 (4 banks
per output group u=j//4, double-buffered = all 8 banks).  The 4 lane
banks are combined with 2 DVE adds (PSUM->SBUF) + 1 GPSIMD add, then
DMA'd to DRAM as yT.  Host transposes/gathers.
"""

import os
import sys
import types
import numpy as np

# ---- problem constants (hardcoded per contract) ----
BLK = 32
KB = 128          # K // BLK
NB = 128          # N // BLK
NCORES = 8
MM_N = 512        # moving free dim per matmul == M_local
NT = KB // 4      # 32 xT tiles of [128, 512]
NU = NB // 4      # 32 output groups of [128, 512]

# "f32": native fp32 matmuls (4 cyc/row). "bf16x3": split-precision
# (x=xh+xl, w=wh+wl in bf16; 3 matmuls/block at 1 cyc/row).
MODE = os.environ.get("SPARSE_KERNEL_MODE", "bf16x3")

LAST_EXEC_TIME_NS = None
LAST_TRACE_DIR = None


def _install_ntff_shim():
    """Best-effort: register the axon NTFF profile hook so trace=True works."""
    try:
        if "antenv.axon_hooks" in sys.modules:
            return True
        import antenv
        mod = types.ModuleType("antenv.axon_hooks")
        mod._hook = None
        mod.set_axon_ntff_profile_hook = lambda h: setattr(mod, "_hook", h)
        mod.get_axon_ntff_profile_hook = lambda: mod._hook
        sys.modules["antenv.axon_hooks"] = mod
        antenv.axon_hooks = mod
        from trn_agent_boot.trn_boot import _ntff_profile_via_ctypes
        hook = _ntff_profile_via_ctypes("/opt/axon/libaxon_pjrt.so")
        if hook is not None:
            mod._hook = hook
        return hook is not None
    except Exception:
        return False


def _build_schedule(idx_i, idx_j):
    """chains[u][r][c] = list of (t, b); slot[b] = weight slot within lane r."""
    nnz = idx_i.shape[0]
    chains = [[[[] for _ in range(4)] for _ in range(4)] for _ in range(NU)]
    for b in range(nnz):
        i = int(idx_i[b])
        j = int(idx_j[b])
        chains[j // 4][i % 4][j % 4].append((i // 4, b))
    # sort each chain by xT tile index so compute can start while x streams in
    slot = np.zeros(nnz, dtype=np.int64)
    counts = [1, 1, 1, 1]  # slot 0 of each lane = zeros (for empty chains)
    for u in range(NU):
        for r in range(4):
            for c in range(4):
                chains[u][r][c].sort()
                for (t, b) in chains[u][r][c]:
                    slot[b] = counts[r]
                    counts[r] += 1
    return chains, slot, max(counts)


def _pack_w(wcomp, idx_i, slot, S):
    """Pack [nnz,32,32] blocks into [128, S*32]: lane r=i%4 partitions, slot s."""
    wp = np.zeros((128, S * BLK), dtype=wcomp.dtype)
    r = (np.asarray(idx_i) % 4).astype(np.int64)
    for b in range(wcomp.shape[0]):
        p0 = 32 * r[b]
        f0 = BLK * slot[b]
        wp[p0:p0 + 32, f0:f0 + 32] = wcomp[b]
    return wp


def _build_program(chains, slot, S, dt_in):
    import concourse.bacc as bacc
    import concourse.tile as tile
    from concourse import mybir
    from concourse.alu_op_type import AluOpType

    f32 = mybir.dt.float32
    split = MODE == "bf16x3"

    nc = bacc.Bacc("TRN2", debug=False, num_devices=NCORES)
    if split:
        x_h_d = nc.dram_tensor("x_h", [KB * BLK, MM_N], dt_in, kind="ExternalInput").ap()
        x_l_d = nc.dram_tensor("x_l", [KB * BLK, MM_N], dt_in, kind="ExternalInput").ap()
        w_h_d = nc.dram_tensor("w_h", [128, S * BLK], dt_in, kind="ExternalInput").ap()
        w_l_d = nc.dram_tensor("w_l", [128, S * BLK], dt_in, kind="ExternalInput").ap()
    else:
        x_d = nc.dram_tensor("x", [KB * BLK, MM_N], dt_in, kind="ExternalInput").ap()
        w_d = nc.dram_tensor("w", [128, S * BLK], dt_in, kind="ExternalInput").ap()
    y_d = nc.dram_tensor("y", [NB * BLK, MM_N], f32, kind="ExternalOutput").ap()

    with tile.TileContext(nc) as tc:
        with tc.tile_pool(name="const", bufs=1) as cpool, \
             tc.tile_pool(name="work", bufs=3) as wpool, \
             tc.tile_pool(name="psum", bufs=2, space="PSUM") as ppool:

            if split:
                x_sb_h = cpool.tile([128, NT * MM_N], dt_in, name="x_sb_h")
                x_sb_l = cpool.tile([128, NT * MM_N], dt_in, name="x_sb_l")
                w_sb_h = cpool.tile([128, S * BLK], dt_in, name="w_sb_h")
                w_sb_l = cpool.tile([128, S * BLK], dt_in, name="w_sb_l")
                xs = [x_sb_h, x_sb_l]
                xds = [x_h_d, x_l_d]
                ws = [w_sb_h, w_sb_l]
                wds = [w_h_d, w_l_d]
            else:
                x_sb = cpool.tile([128, NT * MM_N], dt_in, name="x_sb")
                w_sb = cpool.tile([128, S * BLK], dt_in, name="w_sb")
                xs = [x_sb]
                xds = [x_d]
                ws = [w_sb]
                wds = [w_d]

            # x loads: one DMA per [128, 512] tile so matmuls gate per-tile
            for t in range(NT):
                for sb, d in zip(xs, xds):
                    nc.sync.dma_start(
                        out=sb[:, t * MM_N:(t + 1) * MM_N],
                        in_=d[t * 128:(t + 1) * 128, :])
            # w loads: 8 chunks along the slot axis (slots assigned in u order)
            WCH = 8
            bnds = [round(S * k / WCH) * BLK for k in range(WCH + 1)]
            for k in range(WCH):
                if bnds[k] == bnds[k + 1]:
                    continue
                for sb, d in zip(ws, wds):
                    nc.sync.dma_start(out=sb[:, bnds[k]:bnds[k + 1]],
                                      in_=d[:, bnds[k]:bnds[k + 1]])

            for u in range(NU):
                ps = [ppool.tile([128, MM_N], f32, tag=f"ps{r}", name=f"ps_u{u}_r{r}")
                      for r in range(4)]
                for r in range(4):
                    for c in range(4):
                        blocks = chains[u][r][c]
                        out_ap = ps[r][32 * c:32 * c + 32, :]
                        tp = (32 * r, 32 * c)
                        if not blocks:
                            # zero block: defines the psum region
                            nc.tensor.matmul(
                                out=out_ap,
                                lhsT=ws[0][32 * r:32 * r + 32, 0:BLK],
                                rhs=xs[0][32 * r:32 * r + 32, 0:MM_N],
                                start=True, stop=True, tile_position=tp)
                            continue
                        nmm = len(blocks) * (3 if split else 1)
                        k = 0
                        for (t, b) in blocks:
                            s = int(slot[b])
                            wsl = [sb[32 * r:32 * r + 32, BLK * s:BLK * s + 32]
                                   for sb in ws]
                            xsl = [sb[32 * r:32 * r + 32, t * MM_N:(t + 1) * MM_N]
                                   for sb in xs]
                            if split:
                                terms = [(wsl[0], xsl[0]), (wsl[1], xsl[0]),
                                         (wsl[0], xsl[1])]
                            else:
                                terms = [(wsl[0], xsl[0])]
                            for (wt, xt) in terms:
                                nc.tensor.matmul(
                                    out=out_ap, lhsT=wt, rhs=xt,
                                    start=(k == 0), stop=(k == nmm - 1),
                                    tile_position=tp)
                                k += 1

                # combine the 4 lane banks; a DVE/ACT op may read at most one
                # PSUM operand, so: ACT copies ps0/ps2 to SBUF, DVE adds
                # ps1/ps3 onto them, GPSIMD does the final SBUF-only add.
                cp0 = wpool.tile([128, MM_N], f32, tag="cp0", name=f"cp0_u{u}")
                cp1 = wpool.tile([128, MM_N], f32, tag="cp1", name=f"cp1_u{u}")
                acc0 = wpool.tile([128, MM_N], f32, tag="acc0", name=f"acc0_u{u}")
                acc1 = wpool.tile([128, MM_N], f32, tag="acc1", name=f"acc1_u{u}")
                yt = wpool.tile([128, MM_N], f32, tag="yt", name=f"yt_u{u}")
                nc.scalar.copy(out=cp0, in_=ps[0])
                nc.scalar.copy(out=cp1, in_=ps[2])
                nc.vector.tensor_tensor(out=acc0, in0=cp0, in1=ps[1],
                                        op=AluOpType.add)
                nc.vector.tensor_tensor(out=acc1, in0=cp1, in1=ps[3],
                                        op=AluOpType.add)
                nc.gpsimd.tensor_tensor(out=yt, in0=acc0, in1=acc1,
                                        op=AluOpType.add)
                nc.sync.dma_start(out=y_d[u * 128:(u + 1) * 128, :], in_=yt)

    nc.compile()
    return nc


def kernel(x, w, idx_i, idx_j):
    global LAST_EXEC_TIME_NS, LAST_TRACE_DIR
    from concourse import mybir
    from concourse.bass_utils import run_bass_kernel_spmd

    x = np.asarray(x, dtype=np.float32)
    w = np.asarray(w, dtype=np.float32)
    idx_i = np.asarray(idx_i, dtype=np.int32)
    idx_j = np.asarray(idx_j, dtype=np.int32)
    M = x.shape[0]
    assert M % NCORES == 0 and M // NCORES == MM_N

    chains, slot, S = _build_schedule(idx_i, idx_j)

    xT = np.ascontiguousarray(x.T)  # [K, M]

    if MODE == "bf16x3":
        import ml_dtypes
        bf16 = ml_dtypes.bfloat16
        dt_in = mybir.dt.bfloat16
        w_h = w.astype(bf16)
        w_l = (w - w_h.astype(np.float32)).astype(bf16)
        wp_h = _pack_w(w_h, idx_i, slot, S)
        wp_l = _pack_w(w_l, idx_i, slot, S)
        xT_h = xT.astype(bf16)
        xT_l = (xT - xT_h.astype(np.float32)).astype(bf16)
        nc = _build_program(chains, slot, S, dt_in)
        in_maps = []
        for cid in range(NCORES):
            sl = np.s_[:, cid * MM_N:(cid + 1) * MM_N]
            in_maps.append({
                "x_h": np.ascontiguousarray(xT_h[sl]),
                "x_l": np.ascontiguousarray(xT_l[sl]),
                "w_h": wp_h, "w_l": wp_l,
            })
    else:
        dt_in = mybir.dt.float32
        wp = _pack_w(w, idx_i, slot, S)
        nc = _build_program(chains, slot, S, dt_in)
        in_maps = []
        for cid in range(NCORES):
            sl = np.s_[:, cid * MM_N:(cid + 1) * MM_N]
            in_maps.append({"x": np.ascontiguousarray(xT[sl]), "w": wp})

    trace = bool(os.environ.get("SPARSE_KERNEL_PROFILE"))
    if trace:
        trace = _install_ntff_shim()
    tmpdir = os.environ.get("SPARSE_KERNEL_TRACE_DIR") or None
    res = run_bass_kernel_spmd(nc, in_maps, core_ids=list(range(NCORES)),
                               trace=trace, tmpdir=tmpdir)
    LAST_EXEC_TIME_NS = res.exec_time_ns
    LAST_TRACE_DIR = tmpdir

    y = np.empty((M, NB * BLK), dtype=np.float32)
    for cid in range(NCORES):
        y[cid * MM_N:(cid + 1) * MM_N, :] = res.results[cid]["y"].T
    return y
